# revision 42
# baseline (speedup 1.0000x reference)
"""GQA attention block (RMSNorm-QK, causal, GQA) on 8 trn2 NeuronCores.

Strategy: strided sequence sharding, zero collectives. Core c handles batch
c//4 and queries at positions j::4 (j = c%4) of that batch, keys in natural
order. With stride-4 queries sorted ascending, causality is *uniform* across
cores: for key tile kt, exactly the query columns [32*kt, 512) are (at least
partially) valid, independent of j. Scores/exp/den/context matmuls are all
sliced to that suffix (~47% less attention work than full S); the only
j-dependent data is a single [128, 32] diagonal band mask from the host.

Activations and weights stream as bf16 (fp32 PSUM accumulation) — halves the
HBM traffic, which otherwise co-bottlenecks with the PE; weights are host-
swizzled so every DMA lands as one contiguous 4-8KB segment per partition.
All activations are feature-major; V is produced directly in [token, d]
layout by swapping matmul operands, so the kernel has zero on-device
transposes. Softmax needs no max subtraction (RMS-normalized q,k bound
|scores|/sqrt(D) <= sqrt(D)). Softmax denominators are rank-1 PE matmuls;
1/den uses the fast custom-DVE reciprocal, broadcast across partitions on
the (otherwise idle) GpSimd engine.
"""

import math
import os
import numpy as np
import ml_dtypes
from contextlib import ExitStack

import concourse.bass as bass
import concourse.mybir as mybir
import concourse.tile as tile
from concourse import bacc
from concourse.bass_utils import run_bass_kernel_spmd

F32 = mybir.dt.float32
F32R = mybir.dt.float32r
BF16 = mybir.dt.bfloat16
NPBF = ml_dtypes.bfloat16
ADD = mybir.AluOpType.add
MULT = mybir.AluOpType.mult
EXP = mybir.ActivationFunctionType.Exp
SQRT = mybir.ActivationFunctionType.Sqrt

EPS = 1e-8


def full_cfg():
    return dict(B=2, S=2048, E=2048, D=128, G=2)


def derived(cfg):
    B, S, E, D, G = cfg["B"], cfg["S"], cfg["E"], cfg["D"], cfg["G"]
    NH = E // D            # query heads (16)
    ET = E // 128          # 128-row tiles of E (16)
    NKT = S // 128         # key tiles (16)
    QPC = S // 4           # query tokens per core (512)
    GS = NH // G           # heads per kv group (8)
    assert D == 128 and QPC == 512
    return NH, ET, NKT, QPC, GS


def build_program(cfg):
    B, S, E, D, G = cfg["B"], cfg["S"], cfg["E"], cfg["D"], cfg["G"]
    NH, ET, NKT, QPC, GS = derived(cfg)
    SCALE = 1.0 / math.sqrt(D)
    KC = 512               # token-column chunk width for K/V projections
    NKC = S // KC          # 4
    GRP = 3                # key tiles per exp group (3 PSUM banks)
    NGRP = (NKT + GRP - 1) // GRP  # 6 (last group has 1)

    def co(kt):            # first computed query column for key tile kt
        return 32 * kt

    nc = bacc.Bacc()
    # x in [E, S] bf16; DMA'd as [128, 1024] tiles (2KB rows)
    xT_d = nc.dram_tensor("xT", [E, S], BF16, kind="ExternalInput")
    xq_d = nc.dram_tensor("xq", [E, QPC], BF16, kind="ExternalInput")
    # weights host-swizzled: wq_sw[p, qc, et*128+d] = Wq[et*128+p, qc*128+d]
    wq_d = nc.dram_tensor("wq_sw", [128, ET, E], BF16, kind="ExternalInput")
    wo_d = nc.dram_tensor("wo_sw", [128, ET, E], BF16, kind="ExternalInput")
    # wk_sw[p, et*256+c] = Wk[et*128+p, c]
    wk_d = nc.dram_tensor("wk_sw", [128, ET * G * D], BF16, kind="ExternalInput")
    wv_d = nc.dram_tensor("wv_sw", [128, ET * G * D], BF16, kind="ExternalInput")
    bq_d = nc.dram_tensor("bq_t", [128, ET], F32, kind="ExternalInput")
    bk_d = nc.dram_tensor("bk_t", [128, G], F32, kind="ExternalInput")
    bv_d = nc.dram_tensor("bv_b", [128, G * D], F32, kind="ExternalInput")
    bo_d = nc.dram_tensor("bo_t", [128, ET], F32, kind="ExternalInput")
    gq_d = nc.dram_tensor("gq_r", [1, 128], F32, kind="ExternalInput")
    gk_d = nc.dram_tensor("gk_r", [1, 128], F32, kind="ExternalInput")
    mask_d = nc.dram_tensor("mask32", [128, 32], F32, kind="ExternalInput")
    ones_d = nc.dram_tensor("ones1", [128, 1], F32, kind="ExternalInput")
    ones2_d = nc.dram_tensor("ones2", [128, 128], F32, kind="ExternalInput")
    out_d = nc.dram_tensor("outT", [E, QPC], F32, kind="ExternalOutput")
    debug = os.environ.get("KERNEL_DEBUG_DEN", "0") == "1"
    if debug:
        dbgden_d = nc.dram_tensor("dbgden", [NH, QPC], F32, kind="ExternalOutput")
        dbgcx_d = nc.dram_tensor("dbgcx", [128, QPC], F32, kind="ExternalOutput")

    wkr = wk_d.rearrange("p (t c) -> p t c", c=G * D)   # [128, ET, 256]
    wvr = wv_d.rearrange("p (t c) -> p t c", c=G * D)

    with tile.TileContext(nc) as tc, ExitStack() as top:
        consts = top.enter_context(tc.tile_pool(name="consts", bufs=1))
        persist = top.enter_context(tc.tile_pool(name="persist", bufs=1))

        ones_col = consts.tile([128, 1], F32R)
        nc.sync.dma_start(out=ones_col, in_=ones_d[:, :].bitcast(F32R))
        ones_sq = consts.tile([128, 128], F32R)
        nc.sync.dma_start(out=ones_sq, in_=ones2_d[:, :].bitcast(F32R))
        ones_bf = consts.tile([128, 1], BF16)
        nc.vector.memset(ones_bf, 1.0)
        eps_t = consts.tile([1, 1], F32)
        nc.vector.memset(eps_t, EPS)
        gq_sb = consts.tile([1, 128], F32)
        nc.sync.dma_start(out=gq_sb, in_=gq_d[:, :])
        gk_sb = consts.tile([1, 128], F32)
        nc.sync.dma_start(out=gk_sb, in_=gk_d[:, :])
        bq_sb = consts.tile([128, ET], F32)
        nc.sync.dma_start(out=bq_sb, in_=bq_d[:, :])
        bk_sb = consts.tile([128, G], F32)
        nc.sync.dma_start(out=bk_sb, in_=bk_d[:, :])
        bv_sb = consts.tile([128, G * D], F32)
        nc.sync.dma_start(out=bv_sb, in_=bv_d[:, :])
        bo_sb = consts.tile([128, ET], F32)
        nc.sync.dma_start(out=bo_sb, in_=bo_d[:, :])
        mask_sb = consts.tile([128, 32], F32R)
        nc.sync.dma_start(out=mask_sb, in_=mask_d[:, :].bitcast(F32R))

        ktn = [persist.tile([128, S], BF16, tag=f"ktn{g}", name=f"ktn{g}") for g in range(G)]
        vtok = [persist.tile([128, NKT, 128], F32R, tag=f"vtok{g}", name=f"vtok{g}") for g in range(G)]
        qtn = persist.tile([128, NH, QPC], BF16, tag="qtn")

        # ---------------- phase 1+2: projections ------------------------
        with ExitStack() as p12:
            wkvp = p12.enter_context(tc.tile_pool(name="wkv", bufs=1))
            xsp = p12.enter_context(tc.tile_pool(name="xs", bufs=36))
            xqp = p12.enter_context(tc.tile_pool(name="xqp", bufs=1))
            tmp = p12.enter_context(tc.tile_pool(name="tmp12", bufs=3))
            wqp = p12.enter_context(tc.tile_pool(name="wqs", bufs=3))
            pkv = p12.enter_context(tc.tile_pool(name="pkv", bufs=2, space="PSUM"))
            pv = p12.enter_context(tc.tile_pool(name="pv", bufs=4, space="PSUM"))
            pssq = p12.enter_context(tc.tile_pool(name="pssq", bufs=1, space="PSUM"))
            pbc = p12.enter_context(tc.tile_pool(name="pbc", bufs=1, space="PSUM"))

            wk_sb = wkvp.tile([128, ET, G * D], BF16, tag="wk")
            wv_sb = wkvp.tile([128, ET, G * D], BF16, tag="wv")

            pending = []

            def flush():
                while pending:
                    pending.pop(0)()

            # prefetch the strided-query columns during phase 1
            xq = []
            for et in range(ET):
                xt = xqp.tile([128, QPC], BF16, tag=f"xq{et}", name=f"xq{et}")
                nc.sync.dma_start(
                    out=xt, in_=xq_d[et * 128:(et + 1) * 128, :])
                xq.append(xt)

            # x arrives as [128, 1024] tiles (2KB rows); each covers 2 chunks
            xbig = {}
            for kc in range(NKC):
                xts = []
                for et in range(ET):
                    if kc == 0:
                        # interleave weight slices with the first x chunk
                        nc.sync.dma_start(out=wk_sb[:, et, :], in_=wkr[:, et, :])
                        nc.sync.dma_start(out=wv_sb[:, et, :], in_=wvr[:, et, :])
                    if kc % 2 == 0:
                        xb = xsp.tile([128, 2 * KC], BF16, tag="xt")
                        nc.sync.dma_start(
                            out=xb, in_=xT_d[et * 128:(et + 1) * 128,
                                             kc * KC:(kc + 2) * KC])
                        xbig[et] = xb
                        xts.append(xb[:, 0:KC])
                    else:
                        xts.append(xbig[et][:, KC:2 * KC])
                # K projection: [d, tok] layout
                acck = []
                for g in range(G):
                    acc = pkv.tile([128, KC], F32, tag="pkv", name="acck")
                    acck.append(acc)
                for et in range(ET):
                    for g in range(G):
                        nc.tensor.matmul(
                            acck[g],
                            lhsT=wk_sb[:, et, g * D:(g + 1) * D],
                            rhs=xts[et],
                            start=(et == 0), stop=(et == ET - 1))
                # V projection: [tok, d] layout (x tile stationary)
                accv = []
                for s in range(KC // 128):
                    acc = pv.tile([128, G * D], F32, tag="pv", name="accv")
                    accv.append(acc)
                    for et in range(ET):
                        nc.tensor.matmul(
                            acc,
                            lhsT=xts[et][:, s * 128:(s + 1) * 128],
                            rhs=wv_sb[:, et, :],
                            start=(et == 0), stop=(et == ET - 1))
                flush()

                def post_kv(kc=kc, acck=acck, accv=accv):
                    for g in range(G):
                        vb = tmp.tile([128, KC], F32, tag="vb", name="vb")
                        nc.vector.tensor_scalar(
                            out=vb, in0=acck[g], scalar1=bk_sb[:, g:g + 1],
                            scalar2=None, op0=ADD)
                        sq = tmp.tile([128, KC], BF16, tag="sq", name="sq")
                        nc.vector.tensor_tensor(out=sq, in0=vb, in1=vb, op=MULT)
                        ssq = pssq.tile([1, KC], F32, tag="ssq", name="ssq")
                        nc.tensor.matmul(ssq, lhsT=ones_bf, rhs=sq,
                                         start=True, stop=True)
                        rms = tmp.tile([1, KC], F32, tag="rms", name="rms")
                        nc.scalar.activation(out=rms, in_=ssq, func=SQRT,
                                             scale=1.0 / D, bias=eps_t[:, :])
                        rinv = tmp.tile([1, KC], F32, tag="rinv", name="rinv")
                        nc.vector.reciprocal_approx_fast(out=rinv, in_=rms)
                        bc = pbc.tile([128, KC], F32, tag="bc", name="bc")
                        nc.tensor.matmul(bc, lhsT=gk_sb, rhs=rinv,
                                         start=True, stop=True)
                        nc.vector.tensor_tensor(
                            out=ktn[g][:, kc * KC:(kc + 1) * KC],
                            in0=vb, in1=bc, op=MULT)
                    for s in range(KC // 128):
                        kt = kc * (KC // 128) + s
                        for g in range(G):
                            nc.vector.tensor_tensor(
                                out=vtok[g][:, kt, :],
                                in0=accv[s][:, g * D:(g + 1) * D],
                                in1=bv_sb[:, g * D:(g + 1) * D], op=ADD)
                pending.append(post_kv)
            flush()

            # ---- phase 2: Q projection from host-gathered strided cols ----
            for qc in range(NH):
                wq_sb = wqp.tile([128, ET, 128], BF16, tag="wq", name="wq")
                nc.sync.dma_start(
                    out=wq_sb,
                    in_=wq_d[:, qc, :].rearrange("p (t c) -> p t c", c=128))
                acc = pkv.tile([128, QPC], F32, tag="pkv", name="qacc")
                for et in range(ET):
                    nc.tensor.matmul(acc, lhsT=wq_sb[:, et, :], rhs=xq[et],
                                     start=(et == 0), stop=(et == ET - 1))

                def post_q(qc=qc, acc=acc):
                    vb = tmp.tile([128, QPC], F32, tag="vb", name="qb")
                    nc.vector.tensor_scalar(
                        out=vb, in0=acc, scalar1=bq_sb[:, qc:qc + 1],
                        scalar2=None, op0=ADD)
                    sq = tmp.tile([128, QPC], BF16, tag="sq", name="qsq")
                    nc.vector.tensor_tensor(out=sq, in0=vb, in1=vb, op=MULT)
                    ssq = pssq.tile([1, QPC], F32, tag="ssq", name="qssq")
                    nc.tensor.matmul(ssq, lhsT=ones_bf, rhs=sq,
                                     start=True, stop=True)
                    rms = tmp.tile([1, QPC], F32, tag="rms", name="qrms")
                    nc.scalar.activation(out=rms, in_=ssq, func=SQRT,
                                         scale=1.0 / D, bias=eps_t[:, :])
                    rinv = tmp.tile([1, QPC], F32, tag="rinv", name="qrinv")
                    nc.vector.reciprocal_approx_fast(out=rinv, in_=rms)
                    bc = pbc.tile([128, QPC], F32, tag="bc", name="qbc")
                    nc.tensor.matmul(bc, lhsT=gq_sb, rhs=rinv,
                                     start=True, stop=True)
                    nc.vector.tensor_tensor(out=qtn[:, qc, :], in0=vb,
                                            in1=bc, op=MULT)
                pending.append(post_q)
                if qc >= 1:
                    pending.pop(0)()
            flush()

        # ---------------- phase 3: attention + phase 4: out proj --------
        with ExitStack() as p34:
            ctxp = p34.enter_context(tc.tile_pool(name="ctxp", bufs=1))
            ctxt = ctxp.tile([128, ET, QPC], BF16, tag="ctxt", name="ctxt")
            ptp = p34.enter_context(tc.tile_pool(name="pt", bufs=1))
            smal = p34.enter_context(tc.tile_pool(name="smal", bufs=1))
            wop = p34.enter_context(tc.tile_pool(name="wos", bufs=3))
            osb = p34.enter_context(tc.tile_pool(name="osb", bufs=3))
            psc = p34.enter_context(tc.tile_pool(name="psc", bufs=2, space="PSUM"))
            pcx = p34.enter_context(tc.tile_pool(name="pcx", bufs=1, space="PSUM"))
            pdn = p34.enter_context(tc.tile_pool(name="pdn", bufs=1, space="PSUM"))

            pending2 = []

            def flush2():
                while pending2:
                    pending2.pop(0)()

            for h in range(NH):
                g_kv = h // GS
                cx = pcx.tile([128, QPC], F32, tag="cx", name="cx")
                dnb = pdn.tile([128, QPC], F32, tag="dnb", name="dnb")
                for grp in range(NGRP):
                    kts = list(range(grp * GRP, min((grp + 1) * GRP, NKT)))
                    c0g = co(kts[0])
                    sct = psc.tile([128, GRP, QPC], F32, tag="sc", name="sct")
                    ptt = ptp.tile([128, GRP, QPC], F32R, tag="pt", name="ptt",
                                   bufs=6)
                    for i, kt in enumerate(kts):
                        nc.tensor.matmul(
                            sct[:, i, c0g:QPC],
                            lhsT=ktn[g_kv][:, kt * 128:(kt + 1) * 128],
                            rhs=qtn[:, h, c0g:QPC],
                            start=True, stop=True)

                    def post_grp(h=h, g_kv=g_kv, kts=kts, c0g=c0g,
                                 sct=sct, ptt=ptt, cx=cx, dnb=dnb):
                        n = len(kts)
                        nc.scalar.activation(
                            out=ptt[:, 0:n, c0g:QPC], in_=sct[:, 0:n, c0g:QPC],
                            func=EXP, scale=SCALE)
                        for i, kt in enumerate(kts):
                            nc.vector.tensor_tensor(
                                out=ptt[:, i, 32 * kt:32 * kt + 32],
                                in0=ptt[:, i, 32 * kt:32 * kt + 32],
                                in1=mask_sb, op=MULT)
                            nc.tensor.matmul(
                                dnb[:, co(kt):QPC], lhsT=ones_sq,
                                rhs=ptt[:, i, co(kt):QPC],
                                start=(kt == 0), stop=(kt == NKT - 1))
                            nc.tensor.matmul(
                                cx[:, co(kt):QPC],
                                lhsT=vtok[g_kv][:, kt, :],
                                rhs=ptt[:, i, co(kt):QPC],
                                start=(kt == 0), stop=(kt == NKT - 1))
                    pending2.append(post_grp)
                    while len(pending2) > 2:
                        pending2.pop(0)()

                def post_head(h=h, cx=cx, dnb=dnb):
                    if debug:
                        dsb = smal.tile([1, QPC], F32, tag=f"dbg{h}", name=f"dbg{h}")
                        nc.vector.tensor_copy(out=dsb, in_=dnb[0:1, :])
                        nc.sync.dma_start(out=dbgden_d[h:h + 1, :], in_=dsb)
                        if h == 0:
                            csb = smal.tile([128, QPC], F32, tag="dbgc", name="dbgc")
                            nc.vector.tensor_copy(out=csb, in_=cx)
                            nc.sync.dma_start(out=dbgcx_d[:, :], in_=csb)
                    rdb = smal.tile([128, QPC], F32, tag="rdb", name="rdb",
                                    bufs=2)
                    nc.vector.reciprocal_approx_fast(out=rdb, in_=dnb)
                    nc.vector.tensor_tensor(out=ctxt[:, h, :], in0=cx,
                                            in1=rdb, op=MULT)
                pending2.append(post_head)
            flush2()

            for c2 in range(ET):
                wo_sb = wop.tile([128, ET, 128], BF16, tag="wo", name="wo")
                nc.sync.dma_start(
                    out=wo_sb,
                    in_=wo_d[:, c2, :].rearrange("p (t c) -> p t c", c=128))
                opool = pcx if c2 % 2 == 0 else pdn
                acc = opool.tile([128, QPC], F32,
                                 tag="cx" if c2 % 2 == 0 else "dnb",
                                 name="oacc")
                for ct in range(ET):
                    nc.tensor.matmul(acc, lhsT=wo_sb[:, ct, :],
                                     rhs=ctxt[:, ct, :],
                                     start=(ct == 0), stop=(ct == ET - 1))

                def post_o(c2=c2, acc=acc):
                    ot = osb.tile([128, QPC], F32, tag="ot", name="ot")
                    nc.vector.tensor_scalar(
                        out=ot, in0=acc, scalar1=bo_sb[:, c2:c2 + 1],
                        scalar2=None, op0=ADD)
                    nc.sync.dma_start(
                        out=out_d[c2 * 128:(c2 + 1) * 128, :], in_=ot)
                pending2.append(post_o)
                while len(pending2) > 2:
                    pending2.pop(0)()
            flush2()
    nc.compile()
    return nc


# ---------------------------------------------------------------------------
# host-side sharding
# ---------------------------------------------------------------------------

def band_mask(j):
    """[128, 32] multiplicative mask for the diagonal key tile band.

    Query col c of the 32-wide band maps to position j + 4*(32*kt + c);
    key row r maps to 128*kt + r: invalid iff r > j + 4c (kt cancels).
    """
    rr = np.arange(128)[:, None]
    cc = np.arange(32)[None, :]
    return (rr <= j + 4 * cc).astype(np.float32)


def swizzle_w(w, E, ET):
    """[E, E] -> [128, ET(out-block), ET(in-block)*128] bf16, contiguous
    per-partition segments for each out-block slice."""
    return np.ascontiguousarray(
        w.reshape(ET, 128, ET, 128).transpose(1, 2, 0, 3)
        .reshape(128, ET, E).astype(NPBF))


def make_in_maps(cfg, inputs):
    B, S, E, D, G = cfg["B"], cfg["S"], cfg["E"], cfg["D"], cfg["G"]
    NH, ET, NKT, QPC, GS = derived(cfg)
    x = np.asarray(inputs["x"], np.float32)
    wk = np.asarray(inputs["Wk"], np.float32)
    wv = np.asarray(inputs["Wv"], np.float32)
    shared = dict(
        wq_sw=swizzle_w(np.asarray(inputs["Wq"], np.float32), E, ET),
        wo_sw=swizzle_w(np.asarray(inputs["Wo"], np.float32), E, ET),
        wk_sw=np.ascontiguousarray(
            wk.reshape(ET, 128, G * D).transpose(1, 0, 2)
            .reshape(128, ET * G * D).astype(NPBF)),
        wv_sw=np.ascontiguousarray(
            wv.reshape(ET, 128, G * D).transpose(1, 0, 2)
            .reshape(128, ET * G * D).astype(NPBF)),
        bq_t=np.ascontiguousarray(
            np.asarray(inputs["bq"], np.float32).reshape(ET, 128).T),
        bk_t=np.ascontiguousarray(
            np.asarray(inputs["bk"], np.float32).reshape(G, 128).T),
        bv_b=np.ascontiguousarray(np.broadcast_to(
            np.asarray(inputs["bv"], np.float32).reshape(1, G * D),
            (128, G * D))),
        bo_t=np.ascontiguousarray(
            np.asarray(inputs["bo"], np.float32).reshape(ET, 128).T),
        gq_r=np.ascontiguousarray(
            np.asarray(inputs["gamma_q"], np.float32).reshape(1, 128)),
        gk_r=np.ascontiguousarray(
            np.asarray(inputs["gamma_k"], np.float32).reshape(1, 128)),
    )
    xTb = [np.ascontiguousarray(x[b].T.astype(NPBF)) for b in range(B)]
    ones = np.ones((128, 1), np.float32)
    ones2 = np.ones((128, 128), np.float32)
    in_maps = []
    for c in range(8):
        b, j = c // 4, c % 4
        m = dict(shared)
        m["xT"] = xTb[b]
        m["xq"] = np.ascontiguousarray(xTb[b][:, j::4])
        m["mask32"] = band_mask(j)
        m["ones1"] = ones
        m["ones2"] = ones2
        in_maps.append(m)
    return in_maps, None


def assemble(cfg, results, perms):
    B, S, E = cfg["B"], cfg["S"], cfg["E"]
    out = np.empty((B, S, E), np.float32)
    for c in range(8):
        b, j = c // 4, c % 4
        out[b, j::4, :] = results[c]["outT"].T
    return out


_CACHE = {}


def kernel(**inputs):
    cfg = full_cfg()
    if "nc" not in _CACHE:
        _CACHE["nc"] = build_program(cfg)
    nc = _CACHE["nc"]
    in_maps, perms = make_in_maps(cfg, inputs)
    res = run_bass_kernel_spmd(nc, in_maps, list(range(8)))
    return assemble(cfg, res.results, perms)


# revision 43
# speedup vs baseline: 1.0261x; 1.0261x over previous
"""GQA attention block (RMSNorm-QK, causal, GQA) on 8 trn2 NeuronCores.

Strategy: strided sequence sharding, zero collectives. Core c handles batch
c//4 and queries at positions j::4 (j = c%4) of that batch, keys in natural
order. With stride-4 queries sorted ascending, causality is *uniform* across
cores: for key tile kt, exactly the query columns [32*kt, 512) are (at least
partially) valid, independent of j. Scores/exp/den/context matmuls are all
sliced to that suffix (~47% less attention work than full S); the only
j-dependent data is a single [128, 32] diagonal band mask from the host.

Activations and weights stream as bf16 (fp32 PSUM accumulation) — halves the
HBM traffic, which otherwise co-bottlenecks with the PE; weights are host-
swizzled so every DMA lands as one contiguous 4-8KB segment per partition.
All activations are feature-major; V is produced directly in [token, d]
layout by swapping matmul operands, so the kernel has zero on-device
transposes. Softmax needs no max subtraction (RMS-normalized q,k bound
|scores|/sqrt(D) <= sqrt(D)). Softmax denominators are rank-1 PE matmuls;
1/den uses the fast custom-DVE reciprocal, broadcast across partitions on
the (otherwise idle) GpSimd engine.
"""

import math
import os
import numpy as np
import ml_dtypes
from contextlib import ExitStack

import concourse.bass as bass
import concourse.mybir as mybir
import concourse.tile as tile
from concourse import bacc
from concourse.bass_utils import run_bass_kernel_spmd

F32 = mybir.dt.float32
F32R = mybir.dt.float32r
BF16 = mybir.dt.bfloat16
NPBF = ml_dtypes.bfloat16
ADD = mybir.AluOpType.add
MULT = mybir.AluOpType.mult
EXP = mybir.ActivationFunctionType.Exp
SQRT = mybir.ActivationFunctionType.Sqrt

EPS = 1e-8


def full_cfg():
    return dict(B=2, S=2048, E=2048, D=128, G=2)


def derived(cfg):
    B, S, E, D, G = cfg["B"], cfg["S"], cfg["E"], cfg["D"], cfg["G"]
    NH = E // D            # query heads (16)
    ET = E // 128          # 128-row tiles of E (16)
    NKT = S // 128         # key tiles (16)
    QPC = S // 4           # query tokens per core (512)
    GS = NH // G           # heads per kv group (8)
    assert D == 128 and QPC == 512
    return NH, ET, NKT, QPC, GS


def build_program(cfg):
    B, S, E, D, G = cfg["B"], cfg["S"], cfg["E"], cfg["D"], cfg["G"]
    NH, ET, NKT, QPC, GS = derived(cfg)
    SCALE = 1.0 / math.sqrt(D)
    KC = 512               # token-column chunk width for K/V projections
    NKC = S // KC          # 4
    GRP = 3                # key tiles per exp group (3 PSUM banks)
    NGRP = (NKT + GRP - 1) // GRP  # 6 (last group has 1)

    def co(kt):            # first computed query column for key tile kt
        return 32 * kt

    nc = bacc.Bacc()
    # x in [E, S] bf16; DMA'd as [128, 1024] tiles (2KB rows)
    xT_d = nc.dram_tensor("xT", [E, S], BF16, kind="ExternalInput")
    xq_d = nc.dram_tensor("xq", [E, QPC], BF16, kind="ExternalInput")
    # weights host-swizzled: wq_sw[p, qc, et*128+d] = Wq[et*128+p, qc*128+d]
    wq_d = nc.dram_tensor("wq_sw", [128, ET, E], BF16, kind="ExternalInput")
    wo_d = nc.dram_tensor("wo_sw", [128, ET, E], BF16, kind="ExternalInput")
    # wk_sw[p, et*256+c] = Wk[et*128+p, c]
    wk_d = nc.dram_tensor("wk_sw", [128, ET * G * D], BF16, kind="ExternalInput")
    wv_d = nc.dram_tensor("wv_sw", [128, ET * G * D], BF16, kind="ExternalInput")
    bq_d = nc.dram_tensor("bq_t", [128, ET], F32, kind="ExternalInput")
    bk_d = nc.dram_tensor("bk_t", [128, G], F32, kind="ExternalInput")
    bv_d = nc.dram_tensor("bv_b", [128, G * D], F32, kind="ExternalInput")
    bo_d = nc.dram_tensor("bo_t", [128, ET], F32, kind="ExternalInput")
    gq_d = nc.dram_tensor("gq_r", [1, 128], F32, kind="ExternalInput")
    gk_d = nc.dram_tensor("gk_r", [1, 128], F32, kind="ExternalInput")
    mask_d = nc.dram_tensor("mask32", [128, 32], F32, kind="ExternalInput")
    ones_d = nc.dram_tensor("ones1", [128, 1], F32, kind="ExternalInput")
    ones2_d = nc.dram_tensor("ones2", [128, 128], F32, kind="ExternalInput")
    out_d = nc.dram_tensor("outT", [E, QPC], F32, kind="ExternalOutput")
    debug = os.environ.get("KERNEL_DEBUG_DEN", "0") == "1"
    if debug:
        dbgden_d = nc.dram_tensor("dbgden", [NH, QPC], F32, kind="ExternalOutput")
        dbgcx_d = nc.dram_tensor("dbgcx", [128, QPC], F32, kind="ExternalOutput")

    wkr = wk_d.rearrange("p (t c) -> p t c", c=G * D)   # [128, ET, 256]
    wvr = wv_d.rearrange("p (t c) -> p t c", c=G * D)

    with tile.TileContext(nc) as tc, ExitStack() as top:
        consts = top.enter_context(tc.tile_pool(name="consts", bufs=1))
        persist = top.enter_context(tc.tile_pool(name="persist", bufs=1))

        ones_col = consts.tile([128, 1], F32R)
        nc.sync.dma_start(out=ones_col, in_=ones_d[:, :].bitcast(F32R))
        ones_sq = consts.tile([128, 128], F32R)
        nc.sync.dma_start(out=ones_sq, in_=ones2_d[:, :].bitcast(F32R))
        ones_bf = consts.tile([128, 1], BF16)
        nc.vector.memset(ones_bf, 1.0)
        eps_t = consts.tile([1, 1], F32)
        nc.vector.memset(eps_t, EPS)
        gq_sb = consts.tile([1, 128], F32)
        nc.sync.dma_start(out=gq_sb, in_=gq_d[:, :])
        gk_sb = consts.tile([1, 128], F32)
        nc.sync.dma_start(out=gk_sb, in_=gk_d[:, :])
        bq_sb = consts.tile([128, ET], F32)
        nc.sync.dma_start(out=bq_sb, in_=bq_d[:, :])
        bk_sb = consts.tile([128, G], F32)
        nc.sync.dma_start(out=bk_sb, in_=bk_d[:, :])
        bv_sb = consts.tile([128, G * D], F32)
        nc.sync.dma_start(out=bv_sb, in_=bv_d[:, :])
        bo_sb = consts.tile([128, ET], F32)
        nc.sync.dma_start(out=bo_sb, in_=bo_d[:, :])
        mask_sb = consts.tile([128, 32], F32R)
        nc.sync.dma_start(out=mask_sb, in_=mask_d[:, :].bitcast(F32R))

        ktn = [persist.tile([128, S], BF16, tag=f"ktn{g}", name=f"ktn{g}") for g in range(G)]
        vtok = [persist.tile([128, NKT, 128], F32R, tag=f"vtok{g}", name=f"vtok{g}") for g in range(G)]
        qtn = persist.tile([128, NH, QPC], BF16, tag="qtn")

        # ---------------- phase 1+2: projections ------------------------
        with ExitStack() as p12:
            wkvp = p12.enter_context(tc.tile_pool(name="wkv", bufs=1))
            xsp = p12.enter_context(tc.tile_pool(name="xs", bufs=36))
            xqp = p12.enter_context(tc.tile_pool(name="xqp", bufs=1))
            tmp = p12.enter_context(tc.tile_pool(name="tmp12", bufs=3))
            wqp = p12.enter_context(tc.tile_pool(name="wqs", bufs=3))
            pkv = p12.enter_context(tc.tile_pool(name="pkv", bufs=2, space="PSUM"))
            pv = p12.enter_context(tc.tile_pool(name="pv", bufs=4, space="PSUM"))
            pssq = p12.enter_context(tc.tile_pool(name="pssq", bufs=1, space="PSUM"))
            pbc = p12.enter_context(tc.tile_pool(name="pbc", bufs=1, space="PSUM"))

            wk_sb = wkvp.tile([128, ET, G * D], BF16, tag="wk")
            wv_sb = wkvp.tile([128, ET, G * D], BF16, tag="wv")

            pending = []

            def flush():
                while pending:
                    pending.pop(0)()

            # x arrives as [128, 1024] tiles (2KB rows); each covers 2 chunks
            xq = []
            xbig = {}
            for kc in range(NKC):
                xts = []
                for et in range(ET):
                    if kc == 0:
                        # interleave weight slices with the first x chunk
                        nc.sync.dma_start(out=wk_sb[:, et, :], in_=wkr[:, et, :])
                        nc.sync.dma_start(out=wv_sb[:, et, :], in_=wvr[:, et, :])
                    if kc % 2 == 0:
                        xb = xsp.tile([128, 2 * KC], BF16, tag="xt")
                        nc.sync.dma_start(
                            out=xb, in_=xT_d[et * 128:(et + 1) * 128,
                                             kc * KC:(kc + 2) * KC])
                        xbig[et] = xb
                        xts.append(xb[:, 0:KC])
                    else:
                        xts.append(xbig[et][:, KC:2 * KC])
                if kc == 1:
                    # prefetch the strided-query columns behind chunk 0
                    for et in range(ET):
                        xt = xqp.tile([128, QPC], BF16, tag=f"xq{et}",
                                      name=f"xq{et}")
                        nc.sync.dma_start(
                            out=xt, in_=xq_d[et * 128:(et + 1) * 128, :])
                        xq.append(xt)
                # K projection: [d, tok] layout
                acck = []
                for g in range(G):
                    acc = pkv.tile([128, KC], F32, tag="pkv", name="acck")
                    acck.append(acc)
                for et in range(ET):
                    for g in range(G):
                        nc.tensor.matmul(
                            acck[g],
                            lhsT=wk_sb[:, et, g * D:(g + 1) * D],
                            rhs=xts[et],
                            start=(et == 0), stop=(et == ET - 1))
                # V projection: [tok, d] layout (x tile stationary)
                accv = []
                for s in range(KC // 128):
                    acc = pv.tile([128, G * D], F32, tag="pv", name="accv")
                    accv.append(acc)
                    for et in range(ET):
                        nc.tensor.matmul(
                            acc,
                            lhsT=xts[et][:, s * 128:(s + 1) * 128],
                            rhs=wv_sb[:, et, :],
                            start=(et == 0), stop=(et == ET - 1))
                flush()

                def post_kv(kc=kc, acck=acck, accv=accv):
                    for g in range(G):
                        vb = tmp.tile([128, KC], F32, tag="vb", name="vb")
                        nc.vector.tensor_scalar(
                            out=vb, in0=acck[g], scalar1=bk_sb[:, g:g + 1],
                            scalar2=None, op0=ADD)
                        sq = tmp.tile([128, KC], BF16, tag="sq", name="sq")
                        nc.vector.tensor_tensor(out=sq, in0=vb, in1=vb, op=MULT)
                        ssq = pssq.tile([1, KC], F32, tag="ssq", name="ssq")
                        nc.tensor.matmul(ssq, lhsT=ones_bf, rhs=sq,
                                         start=True, stop=True)
                        rms = tmp.tile([1, KC], F32, tag="rms", name="rms")
                        nc.scalar.activation(out=rms, in_=ssq, func=SQRT,
                                             scale=1.0 / D, bias=eps_t[:, :])
                        rinv = tmp.tile([1, KC], F32, tag="rinv", name="rinv")
                        nc.vector.reciprocal_approx_fast(out=rinv, in_=rms)
                        bc = pbc.tile([128, KC], F32, tag="bc", name="bc")
                        nc.tensor.matmul(bc, lhsT=gk_sb, rhs=rinv,
                                         start=True, stop=True)
                        nc.vector.tensor_tensor(
                            out=ktn[g][:, kc * KC:(kc + 1) * KC],
                            in0=vb, in1=bc, op=MULT)
                    for s in range(KC // 128):
                        kt = kc * (KC // 128) + s
                        for g in range(G):
                            nc.vector.tensor_tensor(
                                out=vtok[g][:, kt, :],
                                in0=accv[s][:, g * D:(g + 1) * D],
                                in1=bv_sb[:, g * D:(g + 1) * D], op=ADD)
                pending.append(post_kv)
            flush()

            # ---- phase 2: Q projection from host-gathered strided cols ----
            for qc in range(NH):
                wq_sb = wqp.tile([128, ET, 128], BF16, tag="wq", name="wq")
                nc.sync.dma_start(
                    out=wq_sb,
                    in_=wq_d[:, qc, :].rearrange("p (t c) -> p t c", c=128))
                acc = pkv.tile([128, QPC], F32, tag="pkv", name="qacc")
                for et in range(ET):
                    nc.tensor.matmul(acc, lhsT=wq_sb[:, et, :], rhs=xq[et],
                                     start=(et == 0), stop=(et == ET - 1))

                def post_q(qc=qc, acc=acc):
                    vb = tmp.tile([128, QPC], F32, tag="vb", name="qb")
                    nc.vector.tensor_scalar(
                        out=vb, in0=acc, scalar1=bq_sb[:, qc:qc + 1],
                        scalar2=None, op0=ADD)
                    sq = tmp.tile([128, QPC], BF16, tag="sq", name="qsq")
                    nc.vector.tensor_tensor(out=sq, in0=vb, in1=vb, op=MULT)
                    ssq = pssq.tile([1, QPC], F32, tag="ssq", name="qssq")
                    nc.tensor.matmul(ssq, lhsT=ones_bf, rhs=sq,
                                     start=True, stop=True)
                    rms = tmp.tile([1, QPC], F32, tag="rms", name="qrms")
                    nc.scalar.activation(out=rms, in_=ssq, func=SQRT,
                                         scale=1.0 / D, bias=eps_t[:, :])
                    rinv = tmp.tile([1, QPC], F32, tag="rinv", name="qrinv")
                    nc.vector.reciprocal_approx_fast(out=rinv, in_=rms)
                    bc = pbc.tile([128, QPC], F32, tag="bc", name="qbc")
                    nc.tensor.matmul(bc, lhsT=gq_sb, rhs=rinv,
                                     start=True, stop=True)
                    nc.vector.tensor_tensor(out=qtn[:, qc, :], in0=vb,
                                            in1=bc, op=MULT)
                pending.append(post_q)
                if qc >= 1:
                    pending.pop(0)()
            flush()

        # ---------------- phase 3: attention + phase 4: out proj --------
        with ExitStack() as p34:
            ctxp = p34.enter_context(tc.tile_pool(name="ctxp", bufs=1))
            ctxt = ctxp.tile([128, ET, QPC], BF16, tag="ctxt", name="ctxt")
            ptp = p34.enter_context(tc.tile_pool(name="pt", bufs=1))
            smal = p34.enter_context(tc.tile_pool(name="smal", bufs=1))
            wop = p34.enter_context(tc.tile_pool(name="wos", bufs=3))
            osb = p34.enter_context(tc.tile_pool(name="osb", bufs=3))
            psc = p34.enter_context(tc.tile_pool(name="psc", bufs=2, space="PSUM"))
            pcx = p34.enter_context(tc.tile_pool(name="pcx", bufs=1, space="PSUM"))
            pdn = p34.enter_context(tc.tile_pool(name="pdn", bufs=1, space="PSUM"))

            pending2 = []

            def flush2():
                while pending2:
                    pending2.pop(0)()

            for h in range(NH):
                g_kv = h // GS
                cx = pcx.tile([128, QPC], F32, tag="cx", name="cx")
                dnb = pdn.tile([128, QPC], F32, tag="dnb", name="dnb")
                for grp in range(NGRP):
                    kts = list(range(grp * GRP, min((grp + 1) * GRP, NKT)))
                    c0g = co(kts[0])
                    sct = psc.tile([128, GRP, QPC], F32, tag="sc", name="sct")
                    ptt = ptp.tile([128, GRP, QPC], F32R, tag="pt", name="ptt",
                                   bufs=6)
                    for i, kt in enumerate(kts):
                        nc.tensor.matmul(
                            sct[:, i, c0g:QPC],
                            lhsT=ktn[g_kv][:, kt * 128:(kt + 1) * 128],
                            rhs=qtn[:, h, c0g:QPC],
                            start=True, stop=True)

                    def post_grp(h=h, g_kv=g_kv, kts=kts, c0g=c0g,
                                 sct=sct, ptt=ptt, cx=cx, dnb=dnb):
                        n = len(kts)
                        nc.scalar.activation(
                            out=ptt[:, 0:n, c0g:QPC], in_=sct[:, 0:n, c0g:QPC],
                            func=EXP, scale=SCALE)
                        for i, kt in enumerate(kts):
                            nc.vector.tensor_tensor(
                                out=ptt[:, i, 32 * kt:32 * kt + 32],
                                in0=ptt[:, i, 32 * kt:32 * kt + 32],
                                in1=mask_sb, op=MULT)
                            nc.tensor.matmul(
                                dnb[:, co(kt):QPC], lhsT=ones_sq,
                                rhs=ptt[:, i, co(kt):QPC],
                                start=(kt == 0), stop=(kt == NKT - 1))
                            nc.tensor.matmul(
                                cx[:, co(kt):QPC],
                                lhsT=vtok[g_kv][:, kt, :],
                                rhs=ptt[:, i, co(kt):QPC],
                                start=(kt == 0), stop=(kt == NKT - 1))
                    pending2.append(post_grp)
                    while len(pending2) > 2:
                        pending2.pop(0)()

                def post_head(h=h, cx=cx, dnb=dnb):
                    if debug:
                        dsb = smal.tile([1, QPC], F32, tag=f"dbg{h}", name=f"dbg{h}")
                        nc.vector.tensor_copy(out=dsb, in_=dnb[0:1, :])
                        nc.sync.dma_start(out=dbgden_d[h:h + 1, :], in_=dsb)
                        if h == 0:
                            csb = smal.tile([128, QPC], F32, tag="dbgc", name="dbgc")
                            nc.vector.tensor_copy(out=csb, in_=cx)
                            nc.sync.dma_start(out=dbgcx_d[:, :], in_=csb)
                    rdb = smal.tile([128, QPC], F32, tag="rdb", name="rdb",
                                    bufs=2)
                    nc.vector.reciprocal_approx_fast(out=rdb, in_=dnb)
                    nc.vector.tensor_tensor(out=ctxt[:, h, :], in0=cx,
                                            in1=rdb, op=MULT)
                pending2.append(post_head)
            flush2()

            for c2 in range(ET):
                wo_sb = wop.tile([128, ET, 128], BF16, tag="wo", name="wo")
                nc.sync.dma_start(
                    out=wo_sb,
                    in_=wo_d[:, c2, :].rearrange("p (t c) -> p t c", c=128))
                opool = pcx if c2 % 2 == 0 else pdn
                acc = opool.tile([128, QPC], F32,
                                 tag="cx" if c2 % 2 == 0 else "dnb",
                                 name="oacc")
                for ct in range(ET):
                    nc.tensor.matmul(acc, lhsT=wo_sb[:, ct, :],
                                     rhs=ctxt[:, ct, :],
                                     start=(ct == 0), stop=(ct == ET - 1))

                def post_o(c2=c2, acc=acc):
                    ot = osb.tile([128, QPC], F32, tag="ot", name="ot")
                    nc.vector.tensor_scalar(
                        out=ot, in0=acc, scalar1=bo_sb[:, c2:c2 + 1],
                        scalar2=None, op0=ADD)
                    nc.sync.dma_start(
                        out=out_d[c2 * 128:(c2 + 1) * 128, :], in_=ot)
                pending2.append(post_o)
                while len(pending2) > 2:
                    pending2.pop(0)()
            flush2()
    nc.compile()
    return nc


# ---------------------------------------------------------------------------
# host-side sharding
# ---------------------------------------------------------------------------

def band_mask(j):
    """[128, 32] multiplicative mask for the diagonal key tile band.

    Query col c of the 32-wide band maps to position j + 4*(32*kt + c);
    key row r maps to 128*kt + r: invalid iff r > j + 4c (kt cancels).
    """
    rr = np.arange(128)[:, None]
    cc = np.arange(32)[None, :]
    return (rr <= j + 4 * cc).astype(np.float32)


def swizzle_w(w, E, ET):
    """[E, E] -> [128, ET(out-block), ET(in-block)*128] bf16, contiguous
    per-partition segments for each out-block slice."""
    return np.ascontiguousarray(
        w.reshape(ET, 128, ET, 128).transpose(1, 2, 0, 3)
        .reshape(128, ET, E).astype(NPBF))


def make_in_maps(cfg, inputs):
    B, S, E, D, G = cfg["B"], cfg["S"], cfg["E"], cfg["D"], cfg["G"]
    NH, ET, NKT, QPC, GS = derived(cfg)
    x = np.asarray(inputs["x"], np.float32)
    wk = np.asarray(inputs["Wk"], np.float32)
    wv = np.asarray(inputs["Wv"], np.float32)
    shared = dict(
        wq_sw=swizzle_w(np.asarray(inputs["Wq"], np.float32), E, ET),
        wo_sw=swizzle_w(np.asarray(inputs["Wo"], np.float32), E, ET),
        wk_sw=np.ascontiguousarray(
            wk.reshape(ET, 128, G * D).transpose(1, 0, 2)
            .reshape(128, ET * G * D).astype(NPBF)),
        wv_sw=np.ascontiguousarray(
            wv.reshape(ET, 128, G * D).transpose(1, 0, 2)
            .reshape(128, ET * G * D).astype(NPBF)),
        bq_t=np.ascontiguousarray(
            np.asarray(inputs["bq"], np.float32).reshape(ET, 128).T),
        bk_t=np.ascontiguousarray(
            np.asarray(inputs["bk"], np.float32).reshape(G, 128).T),
        bv_b=np.ascontiguousarray(np.broadcast_to(
            np.asarray(inputs["bv"], np.float32).reshape(1, G * D),
            (128, G * D))),
        bo_t=np.ascontiguousarray(
            np.asarray(inputs["bo"], np.float32).reshape(ET, 128).T),
        gq_r=np.ascontiguousarray(
            np.asarray(inputs["gamma_q"], np.float32).reshape(1, 128)),
        gk_r=np.ascontiguousarray(
            np.asarray(inputs["gamma_k"], np.float32).reshape(1, 128)),
    )
    xTb = [np.ascontiguousarray(x[b].T.astype(NPBF)) for b in range(B)]
    ones = np.ones((128, 1), np.float32)
    ones2 = np.ones((128, 128), np.float32)
    in_maps = []
    for c in range(8):
        b, j = c // 4, c % 4
        m = dict(shared)
        m["xT"] = xTb[b]
        m["xq"] = np.ascontiguousarray(xTb[b][:, j::4])
        m["mask32"] = band_mask(j)
        m["ones1"] = ones
        m["ones2"] = ones2
        in_maps.append(m)
    return in_maps, None


def assemble(cfg, results, perms):
    B, S, E = cfg["B"], cfg["S"], cfg["E"]
    out = np.empty((B, S, E), np.float32)
    for c in range(8):
        b, j = c // 4, c % 4
        out[b, j::4, :] = results[c]["outT"].T
    return out


_CACHE = {}


def kernel(**inputs):
    cfg = full_cfg()
    if "nc" not in _CACHE:
        _CACHE["nc"] = build_program(cfg)
    nc = _CACHE["nc"]
    in_maps, perms = make_in_maps(cfg, inputs)
    res = run_bass_kernel_spmd(nc, in_maps, list(range(8)))
    return assemble(cfg, res.results, perms)
